# revision 9
# baseline (speedup 1.0000x reference)
"""Position-attention kernel for Trainium2 (8 NeuronCores, Bass/Tile).

Module: q,k = 1x1 convs to C/8 channels, v = 1x1 conv, attn = softmax(q^T k),
y = v @ attn^T, out = gamma*y + x.  Shapes: B=4, C=512, H=W=64 (N=4096, Cq=64).

Sharding: data-parallel over batch x query-halves -> 8 cores. Core i handles
batch i//2, query positions [h*2048, (h+1)*2048) with h = i%2. Each core
computes full K/V projections for its batch (duplicated across the pair) and
its half of Q, then S^T = k^T q in [key m, query n] layout (so no transposes
are needed anywhere), exp, and y = v @ attn^T via vT-stationary matmuls.

v2 changes vs the first working version:
- x and all conv weights ship as fp8e4m3; the projections run as DoubleRow
  fp8 matmuls (2 channel-tiles contracted per instruction), halving the
  projection matmul count. Weights are pre-scaled (qw,kw x16; vw x64*gamma)
  to clear the fp8 subnormal range; the x16 is undone in the bias drain and
  the x64 cancels against the softmax denominator (see below).
- The softmax denominator is accumulated on the PE: an all-64s [128,2,128]
  fp8 lhsT against e2 gives den = 64*sum(exp) replicated across all 128
  partitions, accumulated in PSUM alongside U. This replaces the per-pair
  DVE adds (84us of DVE) and the gpsimd partition reduce. Since vt carries
  a factor 64*gamma and den carries 64, u*(1/den) yields gamma*y exactly.
- Engine rebalance: v/u PSUM drains moved from ACT to DVE; ACT keeps exp
  (the unavoidable 90us) and the q/k bias drains.
- PSUM: u 4 banks + st 3 bufs + den 1 bank = 8.
"""

import numpy as np
import ml_dtypes

import concourse.bass as bass
import concourse.mybir as mybir
import concourse.tile as tile
from concourse import bacc, bass_isa
from concourse.bass_utils import run_bass_kernel_spmd

BF16 = ml_dtypes.bfloat16
FP8E4 = ml_dtypes.float8_e4m3

B, C, H, W = 4, 512, 64, 64
N = H * W            # 4096 keys per batch
NQ = N // 2          # 2048 queries per core
CQ = C // 8          # 64 q/k channels
P = 128
CT = C // P          # 4 channel tiles
MT = N // P          # 32 key tiles
NCH = 512            # matmul moving-dim chunk
QCH = NQ // NCH      # 4 query chunks per core
KCH = N // NCH       # 8 key chunks
NCORES = 8

F32 = mybir.dt.float32
BF = mybir.dt.bfloat16
F8 = mybir.dt.float8e4
F8E = mybir.dt.float8e5
AF = mybir.ActivationFunctionType
PM = mybir.MatmulPerfMode
LN16 = 2.772588722239781  # exp shift (ln 16): E in fp8e5m2, max logit ~10.9 -> e^8.1 ~ 3300 < 57344
SC_KQ = 16.0         # host pre-scale on qw/kw (fp8 subnormal dodge), undone in drain
SC_V = 64.0          # host pre-scale on vw; cancels against the 64 in den
ST_FP8_DP = False    # DoublePixel is silently dropped by the compiler; keep bf16 q/k

_CACHE = {}


def _build_program():
    # Bacc (not raw Bass): its finalize() runs generate_event_semaphores,
    # which splits multi-semaphore waits — walrus codegen allows only one
    # sync wait per instruction.
    nc = bacc.Bacc()

    xb = nc.declare_dram_parameter("xb", [C, N], F8, isOutput=False)
    xr = nc.declare_dram_parameter("xr", [C, NQ], F32, isOutput=False)
    qw = nc.declare_dram_parameter("qw", [C, CQ], F8, isOutput=False)
    kw = nc.declare_dram_parameter("kw", [C, CQ], F8, isOutput=False)
    vw = nc.declare_dram_parameter("vw", [C, C], F8, isOutput=False)
    qb = nc.declare_dram_parameter("qb", [CQ, 1], F32, isOutput=False)
    kb = nc.declare_dram_parameter("kb", [CQ, 1], F32, isOutput=False)
    out = nc.declare_dram_parameter("out", [C, NQ], F32, isOutput=True)

    with tile.TileContext(nc) as tc:
        with tc.tile_pool(name="consts", bufs=1) as consts:
            x_sb = consts.tile([P, CT * N], F8)        # x[b] as 4 c-tiles side by side
            qw_sb = consts.tile([P, CT * CQ], F8)
            kw_sb = consts.tile([P, CT * CQ], F8)
            vw_sb = consts.tile([P, CT * C], F8)
            qb_sb = consts.tile([CQ, 1], F32)
            kb_sb = consts.tile([CQ, 1], F32)
            xr_sb = consts.tile([P, CT * NQ], F32)     # residual (+ gamma*v_b) slice
            # zero-padded to 128 partitions: st matmuls then run in the same
            # (128,128) PE tiling mode as the DoubleRow U matmuls -> no mode
            # switches in the main loop.
            kq_dt = F8 if ST_FP8_DP else BF
            q_sb = consts.tile([P, NQ], kq_dt)
            k_sb = consts.tile([P, N], kq_dt)
            vt_sb = consts.tile([P, MT * C], F8)       # vT: 32 m-tiles of [128, 512]
            c64_sb = consts.tile([P, 2 * P], F8)       # all-64 lhsT for the den matmul

            # Consolidated input DMAs: one instruction per tensor (rearranged
            # APs cover all 4 c-tiles) — each dma_start costs ~0.6us of
            # sequencer descriptor-gen, so fewer + split across the two HWDGE
            # queues (sync, scalar). x is split so its first 512 columns land
            # before the rest streams in.
            xb_r = xb[:, :].rearrange("(t p) m -> p t m", p=P)
            xsb_r = x_sb.rearrange("p (t m) -> p t m", t=CT)
            kw_r = kw[:, :].rearrange("(t p) o -> p t o", p=P)
            qw_r = qw[:, :].rearrange("(t p) o -> p t o", p=P)
            vw_r = vw[:, :].rearrange("(t p) o -> p t o", p=P)
            nc.sync.dma_start(out=kw_sb.rearrange("p (t o) -> p t o", t=CT), in_=kw_r)
            nc.scalar.dma_start(out=qw_sb.rearrange("p (t o) -> p t o", t=CT), in_=qw_r)
            nc.scalar.dma_start(out=kb_sb, in_=kb[:, :])
            nc.scalar.dma_start(out=qb_sb, in_=qb[:, :])
            nc.sync.dma_start(out=xsb_r[:, :2, :NCH], in_=xb_r[:, :2, :NCH])
            nc.scalar.dma_start(out=xsb_r[:, 2:, :NCH], in_=xb_r[:, 2:, :NCH])
            nc.sync.dma_start(out=xsb_r[:, :2, NCH:NQ], in_=xb_r[:, :2, NCH:NQ])
            nc.scalar.dma_start(out=xsb_r[:, 2:, NCH:NQ], in_=xb_r[:, 2:, NCH:NQ])
            nc.scalar.dma_start(out=vw_sb.rearrange("p (t o) -> p t o", t=CT), in_=vw_r)
            nc.sync.dma_start(out=xsb_r[:, :2, NQ:], in_=xb_r[:, :2, NQ:])
            nc.scalar.dma_start(out=xsb_r[:, 2:, NQ:], in_=xb_r[:, 2:, NQ:])
            # Touch the bias tiles on ACT before the matmul stream: the
            # Activation-with-bias struct only has one sync-wait slot, so the
            # real bias copies must not need a separate DMA wait.
            bias_touch = consts.tile([CQ, 2], F32)
            nc.scalar.activation(bias_touch[:, 0:1], kb_sb, AF.Copy)
            nc.scalar.activation(bias_touch[:, 1:2], qb_sb, AF.Copy)
            ln16_sb = consts.tile([P, 1], F32)
            nc.vector.memset(ln16_sb, -LN16)
            nc.vector.memset(k_sb[CQ:, :], 0.0)
            nc.vector.memset(q_sb[CQ:, :], 0.0)
            nc.vector.memset(c64_sb, SC_V)

            kw3 = kw_sb.rearrange("p (t o) -> p t o", t=CT)
            qw3 = qw_sb.rearrange("p (t o) -> p t o", t=CT)
            vw3 = vw_sb.rearrange("p (t o) -> p t o", t=CT)

            # ---- projections (fp8 DoubleRow: 2 channel-tiles per matmul) ----
            # Emitted in x-column-arrival order: work needing only the first
            # 512 columns first, then 512:2048, then the rest.
            with tc.tile_pool(name="proj_ps", bufs=2, space="PSUM") as proj_ps:
                def k_proj(ch):
                    kp = proj_ps.tile([CQ, NCH], F32, tag="kq", name="kp")
                    for j in range(2):
                        nc.tensor.matmul(
                            kp,
                            lhsT=kw3[:, 2 * j:2 * j + 2, :],
                            rhs=xsb_r[:, 2 * j:2 * j + 2, ch * NCH:(ch + 1) * NCH],
                            start=(j == 0), stop=(j == 1),
                            perf_mode=PM.DoubleRow)
                    nc.scalar.activation(k_sb[:CQ, ch * NCH:(ch + 1) * NCH], kp,
                                         AF.Identity, bias=kb_sb, scale=1.0 / SC_KQ)

                def q_proj(ch):
                    qp = proj_ps.tile([CQ, NCH], F32, tag="kq", name="qp")
                    for j in range(2):
                        nc.tensor.matmul(
                            qp,
                            lhsT=qw3[:, 2 * j:2 * j + 2, :],
                            rhs=xsb_r[:, 2 * j:2 * j + 2, ch * NCH:(ch + 1) * NCH],
                            start=(j == 0), stop=(j == 1),
                            perf_mode=PM.DoubleRow)
                    nc.scalar.activation(q_sb[:CQ, ch * NCH:(ch + 1) * NCH], qp,
                                         AF.Identity, bias=qb_sb, scale=1.0 / SC_KQ)

                def v_proj(mt):
                    vp = proj_ps.tile([P, C], F32, tag="v", name="vp")
                    for j in range(2):
                        nc.tensor.matmul(
                            vp,
                            lhsT=xsb_r[:, 2 * j:2 * j + 2, mt * P:(mt + 1) * P],
                            rhs=vw3[:, 2 * j:2 * j + 2, :],
                            start=(j == 0), stop=(j == 1),
                            perf_mode=PM.DoubleRow)
                    nc.vector.tensor_copy(vt_sb[:, mt * C:(mt + 1) * C], vp)

                k_proj(0); q_proj(0)
                for mt in range(4):
                    v_proj(mt)
                for ch in range(1, 4):
                    k_proj(ch); q_proj(ch)
                for mt in range(4, 16):
                    v_proj(mt)
                for ch in range(4, KCH):
                    k_proj(ch)
                for mt in range(16, MT):
                    v_proj(mt)

            nc.scalar.dma_start(out=xr_sb.rearrange("p (t m) -> p t m", t=CT),
                                in_=xr[:, :].rearrange("(t p) m -> p t m", p=P))

            # ---- attention main loop ----
            with (
                tc.tile_pool(name="u_ps", bufs=1, space="PSUM") as u_ps,
                tc.tile_pool(name="st_ps", bufs=3, space="PSUM") as st_ps,
                tc.tile_pool(name="den_ps", bufs=1, space="PSUM") as den_ps,
                tc.tile_pool(name="e_pool", bufs=8) as e_pool,
                tc.tile_pool(name="fin", bufs=2) as fin,
                tc.tile_pool(name="outp", bufs=4) as outp,
            ):
                c64_l = c64_sb.rearrange("p (j m) -> p j m", j=2)
                for ch in range(QCH):
                    u = u_ps.tile([P, CT * NCH], F32, tag="u", name="u")
                    den = den_ps.tile([P, NCH], F32, tag="den", name="den")
                    qs = q_sb[:, ch * NCH:(ch + 1) * NCH]

                    sts = {}

                    def emit_st(mt, _qs=qs):
                        st = st_ps.tile([P, NCH], F32, tag="st", name="st")
                        nc.tensor.matmul(st, lhsT=k_sb[:, mt * P:(mt + 1) * P],
                                         rhs=_qs, start=True, stop=True,
                                         perf_mode=(PM.DoublePixel
                                                    if ST_FP8_DP else None))
                        sts[mt] = st

                    emit_st(0)
                    emit_st(1)
                    emit_st(2)
                    vt_r = vt_sb.rearrange("p (m c) -> p m c", m=MT)
                    last = ch == QCH - 1
                    for t in range(MT // 2):
                        e2 = e_pool.tile([P, 2, NCH], F8E, tag="e", name="e2")
                        for j in range(2):
                            mt = 2 * t + j
                            nc.scalar.activation(e2[:, j, :], sts.pop(mt), AF.Exp,
                                                 bias=ln16_sb)
                            if mt + 3 < MT:
                                emit_st(mt + 3)
                        fin_t = t == MT // 2 - 1
                        # den += 64 * sum_keys(e2), replicated on all 128
                        # partitions (all-64 lhsT keeps the (128,128) tiling).
                        # On the final iteration den goes first so the
                        # reciprocal overlaps the remaining U matmuls.
                        def emit_den(_t=t, _e2=e2, _fin=fin_t):
                            nc.tensor.matmul(
                                den, lhsT=c64_l, rhs=_e2,
                                start=(_t == 0), stop=_fin,
                                perf_mode=PM.DoubleRow)
                        if fin_t and last:
                            emit_den()
                        for c in range(CT):
                            nc.tensor.matmul(
                                u[:, c * NCH:(c + 1) * NCH],
                                lhsT=vt_r[:, 2 * t:2 * t + 2, c * P:(c + 1) * P],
                                rhs=e2,
                                start=(t == 0), stop=fin_t,
                                perf_mode=PM.DoubleRow)
                        if not (fin_t and last):
                            emit_den()

                    if not last:
                        # Drain U out of PSUM on DVE (ACT is exp-saturated) so
                        # the PE can start the next chunk without waiting.
                        uc = outp.tile([P, CT * NCH], F32, tag="uc", name="uc")
                        nc.vector.tensor_copy(uc, u)
                        u_src = uc
                    else:
                        # Final chunk: nothing follows — DVE reads U straight
                        # from PSUM to shorten the tail.
                        u_src = u
                    rec = fin.tile([P, NCH], F32, tag="rec", name="rec")
                    nc.vector.reciprocal_approx_fast(out=rec, in_=den)
                    rec_b = bass.AP(tensor=rec.tensor, offset=rec.offset,
                                    ap=[rec.ap[0], [0, CT], rec.ap[1]])
                    o = outp.tile([P, CT * NCH], F32, tag="o", name="o")
                    o3 = o.rearrange("p (c n) -> p c n", c=CT)
                    u3 = u_src.rearrange("p (c n) -> p c n", c=CT)
                    xr3 = xr_sb.rearrange("p (c m) -> p c m", c=CT)[
                        :, :, ch * NCH:(ch + 1) * NCH]
                    out_r = out[:, :].rearrange("(c p) n -> p c n", p=P)
                    if not last:
                        nc.vector.tensor_mul(o3, u3, rec_b)
                        nc.vector.tensor_add(o3, o3, xr3)
                        nc.sync.dma_start(
                            out=out_r[:, :, ch * NCH:(ch + 1) * NCH], in_=o3)
                    else:
                        # Pipelined tail: per-c-tile normalize/residual/DMA so
                        # the first output bytes leave while later c-tiles are
                        # still finishing on the PE/DVE.
                        rec_b1 = bass.AP(tensor=rec.tensor, offset=rec.offset,
                                         ap=[rec.ap[0], [0, 1], rec.ap[1]])
                        for c in range(CT):
                            nc.vector.tensor_mul(o3[:, c:c + 1, :],
                                                 u3[:, c:c + 1, :], rec_b1)
                            nc.vector.tensor_add(o3[:, c:c + 1, :],
                                                 o3[:, c:c + 1, :],
                                                 xr3[:, c:c + 1, :])
                            nc.sync.dma_start(
                                out=out_r[:, c:c + 1,
                                          ch * NCH:(ch + 1) * NCH],
                                in_=o3[:, c:c + 1, :])
    nc.finalize()
    return nc


def _get_program():
    if "nc" not in _CACHE:
        _CACHE["nc"] = _build_program()
    return _CACHE["nc"]


def make_in_maps(x, q_w, q_b, k_w, k_b, v_w, v_b, gamma):
    x = np.asarray(x, dtype=np.float32)
    gamma_f = float(np.asarray(gamma).reshape(-1)[0])
    qwT = np.ascontiguousarray(SC_KQ * np.asarray(q_w, np.float32).T).astype(FP8E4)
    kwT = np.ascontiguousarray(SC_KQ * np.asarray(k_w, np.float32).T).astype(FP8E4)
    vwT = np.ascontiguousarray(
        SC_V * gamma_f * np.asarray(v_w, np.float32).T).astype(FP8E4)
    qb_c = np.asarray(q_b, np.float32).reshape(CQ, 1)
    kb_c = np.asarray(k_b, np.float32).reshape(CQ, 1)
    gvb = (gamma_f * np.asarray(v_b, np.float32)).reshape(C, 1)

    xf = x.reshape(B, C, N)
    in_maps = []
    for core in range(NCORES):
        b, h = core // 2, core % 2
        mine = xf[b, :, h * NQ:(h + 1) * NQ]
        other = xf[b, :, (1 - h) * NQ:(2 - h) * NQ]
        x_perm = np.concatenate([mine, other], axis=1)
        in_maps.append({
            "xb": x_perm.astype(FP8E4),
            "xr": np.ascontiguousarray(mine) + gvb,
            "qw": qwT, "kw": kwT, "vw": vwT,
            "qb": qb_c, "kb": kb_c,
        })
    return in_maps


def run(in_maps, **kwargs):
    nc = _get_program()
    return run_bass_kernel_spmd(nc, in_maps, list(range(NCORES)), **kwargs)


def gather(results):
    out = np.empty((B, C, N), dtype=np.float32)
    for core in range(NCORES):
        b, h = core // 2, core % 2
        out[b, :, h * NQ:(h + 1) * NQ] = results[core]["out"]
    return out.reshape(B, C, H, W)


def kernel(x, q_w, q_b, k_w, k_b, v_w, v_b, gamma, **_):
    in_maps = make_in_maps(x, q_w, q_b, k_w, k_b, v_w, v_b, gamma)
    res = run(in_maps)
    return gather(res.results)
